# revision 1
# baseline (speedup 1.0000x reference)
"""Causal multi-head attention on 8 trn2 NeuronCores.

Problem: B=2, S=2048, D=1024, H=16 heads, HD=64. fp32 in/out.

Sharding: 8 cores = 2 (batch) x 4 (head groups of 4 heads).
Each core computes, for its batch b and head group g:
  Q^T,K^T  [256, 2048]  (d on partitions, seq on free)  = W^T-slice x
  V        [2048, 256+ones]  (natural, with a ones column per head)
  per 512-wide q chunk, per head:  S^T[k,q] = K^T.T @ Q^T  (PE, contraction 64,
  2-head row-packed), P~ = exp(S^T/8) (ACT), causal via block skipping +
  one gpsimd affine_select per diagonal block, PV: out^T[d,q] accumulated
  over k tiles with V_aug stationary (m=65; row 64 = softmax denominator).
  Divide by denominator (DVE reciprocal + mul, gpsimd partition_broadcast),
  then O_partial = ctx^T.T @ Wo_rows  [2048, 1024].
Host: sums the 4 head-group partials per batch and adds bo + bv @ Wo
(exact: the bv bias contributes the constant row vector bv @ Wo_g).

Default mode "f16in": x/Wq/Wk/Wv ship as fp16 (halves the dominant DMA
traffic; QKV still accumulates in fp32 PSUM), everything downstream uses
float32r matmuls (full 1-cycle/row PE rate at N>=256, ~14-bit mantissa).

Emission schedule: DMAs ordered so chunk-0 dependencies land first; per
q-chunk [V proj, QK proj, attention] interleaved so ACT exp overlaps the
next chunk's PE projections; all Wo projections emitted last (they fill
PE gaps; keeping them out of the per-chunk stream avoids psum pool-slot
blocking of later projections).

Measured on HW: rel err 5.1e-4 vs fp32 reference; ~264us/iteration
single-core, ~270-300us with all 8 cores active (HBM contention).
"""

import sys

if "/opt/trn_rl_repo" not in sys.path:
    sys.path.insert(0, "/opt/trn_rl_repo")

import numpy as np

import concourse.bacc as bacc
import concourse.bass as bass
import concourse.mybir as mybir
import concourse.tile as tile
from concourse.bass_utils import run_bass_kernel_spmd

B, S, D, H = 2, 2048, 1024, 16
HD = D // H  # 64
N_CORES = 8
HEADS_PER_CORE = H // 4  # 4
DG = HEADS_PER_CORE * HD  # 256 head dims per core
P = 128
CHUNK = 512  # q chunk width
N_KT = S // P  # 16 k tiles
N_CH = S // CHUNK  # 4 q chunks
F32 = mybir.dt.float32

_CACHE = {}


def _mm(dt_name):
    return {"f32r": mybir.dt.float32r, "f16in": mybir.dt.float32r,
            "f32": mybir.dt.float32,
            "bf16": mybir.dt.bfloat16}[dt_name]


def _in_dt(dt_name):
    """dtype for the x / Wq / Wk / Wv inputs (DMA-traffic dominant)."""
    return mybir.dt.float16 if dt_name == "f16in" else _mm(dt_name)


def build_kernel(mm_dt="f32r", unroll=1, ablate=()):
    """Build + compile the per-core SPMD program. unroll>1 wraps the body
    in a hardware loop (for pure device timing measurements)."""
    mdt = _mm(mm_dt)
    idt = _in_dt(mm_dt)

    nc = bacc.Bacc("TRN2", target_bir_lowering=False, debug=False)
    xT_d = nc.dram_tensor("xT", [D, S], idt, kind="ExternalInput")
    wq_d = nc.dram_tensor("wq", [D, DG], idt, kind="ExternalInput")
    wk_d = nc.dram_tensor("wk", [D, DG], idt, kind="ExternalInput")
    wv_d = nc.dram_tensor("wv", [D, DG], idt, kind="ExternalInput")
    wo_d = nc.dram_tensor("wo", [DG, D], mdt, kind="ExternalInput")
    bq_d = nc.dram_tensor("bq", [DG, 1], F32, kind="ExternalInput")
    bk_d = nc.dram_tensor("bk", [DG, 1], F32, kind="ExternalInput")
    o_d = nc.dram_tensor("o", [S, D], F32, kind="ExternalOutput")

    NDT = D // P  # 8 contraction tiles over D
    NMT = DG // P  # 2 m-tiles over the core's head dims (= head pairs)

    with tile.TileContext(nc) as tc:
        def body(_iv=None):
            _body(tc, nc, mdt, idt,
                  xT_d, wq_d, wk_d, wv_d, wo_d, bq_d, bk_d, o_d, NDT, NMT,
                  ablate)

        if unroll > 1:
            with tc.For_i(0, unroll, 1):
                body()
        else:
            body()

    nc.compile()
    return nc


def _body(tc, nc, mdt, idt, xT_d, wq_d, wk_d, wv_d, wo_d, bq_d, bk_d, o_d,
          NDT, NMT, ablate=()):
    import contextlib
    ctx = contextlib.ExitStack()
    with ctx:
        const = ctx.enter_context(tc.tile_pool(name="const", bufs=1))
        sbuf = ctx.enter_context(tc.tile_pool(name="sbuf", bufs=1))
        ptile_p = ctx.enter_context(tc.tile_pool(name="ptile", bufs=8))
        den_p = ctx.enter_context(tc.tile_pool(name="den", bufs=3))
        out_p = ctx.enter_context(tc.tile_pool(name="outp", bufs=3))
        qkv_ps = ctx.enter_context(
            tc.tile_pool(name="qkv_ps", bufs=2, space="PSUM"))
        stp_ps = ctx.enter_context(
            tc.tile_pool(name="stp_ps", bufs=2, space="PSUM"))
        pv_ps = ctx.enter_context(
            tc.tile_pool(name="pv_ps", bufs=2, space="PSUM"))

        # ---- load inputs ------------------------------------------------
        # weights/biases first, then xt in chunk-major slices so chunk-0
        # compute starts early; all loads are queued before any output
        # stores (the HWDGE queue is in-order)
        xt = []
        for i in range(NDT):
            t = const.tile([P, S], idt, tag=f"xt{i}", name=f"xt{i}")
            xt.append(t)
        ws = {}
        for name, d in (("wq", wq_d), ("wk", wk_d), ("wv", wv_d)):
            ws[name] = [const.tile([P, DG], idt, tag=f"{name}{i}",
                                   name=f"{name}{i}") for i in range(NDT)]
        wo = [const.tile([P, D], mdt, tag=f"wo{m}", name=f"wo{m}")
              for m in range(NMT)]
        biases = {(name, m): const.tile([P, 1], F32, tag=f"{name}{m}",
                                        name=f"{name}{m}")
                  for name in ("bq", "bk") for m in range(NMT)}

        def dma_w(name, d):
            for i in range(NDT):
                nc.sync.dma_start(ws[name][i][:],
                                  d.ap()[P * i:P * (i + 1), :])

        def dma_xt(ci):
            csl = slice(CHUNK * ci, CHUNK * (ci + 1))
            for k in range(NDT):
                nc.sync.dma_start(xt[k][:, csl],
                                  xT_d.ap()[P * k:P * (k + 1), csl])

        # order: V(0)+QK(0) deps first, then remaining chunks, wo last
        dma_w("wv", wv_d)
        dma_xt(0)
        dma_w("wq", wq_d)
        dma_w("wk", wk_d)
        for (name, m), t in biases.items():
            d = bq_d if name == "bq" else bk_d
            nc.sync.dma_start(t[:], d.ap()[P * m:P * (m + 1), :])
        for ci in range(1, N_CH):
            dma_xt(ci)
        for m in range(NMT):
            nc.sync.dma_start(wo[m][:], wo_d.ap()[P * m:P * (m + 1), :])

        # ---- V projection (natural layout + ones cols) ------------------
        # vaug[j]: [128, 4*65]; per head h cols h*65..h*65+63 = V, col h*65+64 = 1
        ones_f = const.tile([P, HEADS_PER_CORE], F32, tag="ones_f",
                            name="ones_f")
        nc.vector.memset(ones_f[:], 1.0)
        ones_r = const.tile([P, HEADS_PER_CORE], mdt, tag="ones_r",
                            name="ones_r")
        nc.vector.tensor_copy(ones_r[:], ones_f[:])
        vaug = []
        for j in range(N_KT):
            t = sbuf.tile([P, HEADS_PER_CORE * (HD + 1)], mdt, tag=f"vaug{j}", name=f"vaug{j}")
            vaug.append(t)

        def v_proj(j):
            ps = qkv_ps.tile([P, CHUNK], F32, tag="proj", name="proj")
            for k in range(NDT):
                nc.tensor.matmul(
                    ps[:, 0:DG],
                    xt[k][:, P * j:P * (j + 1)],
                    ws["wv"][k][:],
                    start=(k == 0), stop=(k == NDT - 1))
            dst = vaug[j][:].rearrange("p (h x) -> p h x", h=HEADS_PER_CORE)
            srcp = ps[:, 0:DG].rearrange("p (h x) -> p h x", h=HEADS_PER_CORE)
            nc.vector.tensor_copy(dst[:, :, 0:HD], srcp[:, :, :])
            nc.vector.tensor_copy(
                dst[:, :, HD:HD + 1],
                ones_r[:].rearrange("p (h x) -> p h x", x=1))

        # ---- Q^T / K^T projections (d on partitions) --------------------
        qt, kt = [], []
        for name, lst in (("wq", qt), ("wk", kt)):
            for m in range(NMT):
                t = sbuf.tile([P, S], mdt, tag=f"{name}T{m}", name=f"{name}T{m}")
                lst.append(t)
        def qk_proj(ci):
            for name, lst in (("wq", qt), ("wk", kt)):
                bname = "bq" if name == "wq" else "bk"
                for m in range(NMT):
                    ps = qkv_ps.tile([P, CHUNK], F32, tag="proj", name="proj")
                    for k in range(NDT):
                        nc.tensor.matmul(
                            ps[:],
                            ws[name][k][:, P * m:P * (m + 1)],
                            xt[k][:, CHUNK * ci:CHUNK * (ci + 1)],
                            start=(k == 0), stop=(k == NDT - 1))
                    nc.vector.tensor_scalar_add(
                        lst[m][:, CHUNK * ci:CHUNK * (ci + 1)], ps[:],
                        biases[(bname, m)][:])

        # ---- attention + output projection, per q chunk -----------------
        ctxT = [sbuf.tile([P, S], mdt, tag=f"ctxT{m}", name=f"ctxT{m}") for m in range(NMT)]


        wo_work = []
        for ci in range(N_CH):
            for j in range(4 * ci, 4 * ci + 4):
                v_proj(j)
            qk_proj(ci)
            jmax = 4 * ci + 3  # last valid k tile for this chunk
            qsl = slice(CHUNK * ci, CHUNK * (ci + 1))
            for pair in range(NMT):
                pv = [pv_ps.tile([HD + 1, CHUNK], F32, tag="pv", name="pv")
                      for _ in range(2)]
                for j0 in range(0, jmax + 1, 2):
                    js = [j for j in (j0, j0 + 1) if j <= jmax]
                    nj = len(js)
                    pt = {}
                    for hh in range(2):  # head within pair
                        psl = slice(64 * hh, 64 * (hh + 1))
                        st = stp_ps.tile([P, 2 * CHUNK], F32, tag="stp",
                                         name="stp")
                        for gi, j in enumerate(js):
                            nc.tensor.matmul(
                                st[:, CHUNK * gi:CHUNK * (gi + 1)],
                                kt[pair][psl, P * j:P * (j + 1)],
                                qt[pair][psl, qsl],
                                start=True, stop=True)
                        p_t = ptile_p.tile([P, 2 * CHUNK], mdt, tag="ptile",
                                           name="ptile")
                        if "exp" in ablate:
                            nc.vector.tensor_copy(
                                p_t[:, 0:CHUNK * nj], st[:, 0:CHUNK * nj])
                        else:
                            nc.scalar.activation(
                                p_t[:, 0:CHUNK * nj], st[:, 0:CHUNK * nj],
                                mybir.ActivationFunctionType.Exp,
                                scale=0.125)
                        for gi, j in enumerate(js):
                            if j >= 4 * ci and "mask" not in ablate:
                                dd = j - 4 * ci
                                w = P * (dd + 1)
                                base = CHUNK * gi
                                nc.gpsimd.affine_select(
                                    out=p_t[:, base:base + w],
                                    in_=p_t[:, base:base + w],
                                    compare_op=mybir.AluOpType.is_ge,
                                    fill=0.0, base=-P * dd,
                                    pattern=[[1, w]],
                                    channel_multiplier=-1)
                        pt[hh] = p_t
                    for gi, j in enumerate(js):
                        for hh in range(2):
                            h = 2 * pair + hh
                            nc.tensor.matmul(
                                pv[hh][:],
                                vaug[j][:, (HD + 1) * h:(HD + 1) * (h + 1)],
                                pt[hh][:, CHUNK * gi:CHUNK * (gi + 1)],
                                start=(j == 0), stop=(j == jmax))
                # softmax denominator divide; write ctx^T chunk
                # (partition_broadcast only reaches partitions 0-63, so use
                # a base-0 tile per head; DVE ops allow mismatched bases)
                if "div" in ablate:
                    for hh in range(2):
                        nc.vector.tensor_copy(
                            ctxT[pair][64 * hh:64 * (hh + 1), qsl],
                            pv[hh][0:HD, :])
                else:
                    den_t = den_p.tile([1, 2 * CHUNK], F32, tag="den",
                                       name="den")
                    for hh in range(2):
                        nc.vector.tensor_copy(
                            den_t[0:1, CHUNK * hh:CHUNK * (hh + 1)],
                            pv[hh][HD:HD + 1, :])
                    nc.vector.reciprocal(den_t[:], den_t[:])
                    for hh in range(2):
                        recb = den_p.tile([HD, CHUNK], F32,
                                          tag=f"recb{hh}", name=f"recb{hh}")
                        nc.gpsimd.partition_broadcast(
                            recb[0:HD, :],
                            den_t[0:1, CHUNK * hh:CHUNK * (hh + 1)])
                        nc.vector.tensor_mul(
                            ctxT[pair][64 * hh:64 * (hh + 1), qsl],
                            pv[hh][0:HD, :],
                            recb[0:HD, :])
            wo_work.append(ci)

        # ---- Wo projections, emitted last (uses idle PE slots) ----------
        for ci in wo_work:
            for qi in range(4):
                i = 4 * ci + qi
                ot = out_p.tile([P, D], F32, tag="ot", name="ot")
                for e in range(2):
                    ps = qkv_ps.tile([P, CHUNK], F32, tag="proj", name="proj")
                    for m in range(NMT):
                        nc.tensor.matmul(
                            ps[:],
                            ctxT[m][:, P * i:P * (i + 1)],
                            wo[m][:, CHUNK * e:CHUNK * (e + 1)],
                            start=(m == 0), stop=(m == NMT - 1))
                    nc.any.tensor_copy(ot[:, CHUNK * e:CHUNK * (e + 1)],
                                       ps[:])
                nc.sync.dma_start(o_d.ap()[P * i:P * (i + 1), :], ot[:])


def _shard_inputs(x, Wq, bq, Wk, bk, Wv, bv, Wo, bo):
    mm_dt = _CACHE.get("mm_dt", "f16in")
    ndt = np.float16 if mm_dt == "f16in" else np.float32
    x = np.asarray(x, np.float32)
    in_maps = []
    for core in range(N_CORES):
        b, g = divmod(core, 4)
        ds = slice(DG * g, DG * (g + 1))
        in_maps.append({
            "xT": np.ascontiguousarray(x[b].T).astype(ndt),
            "wq": np.ascontiguousarray(
                np.asarray(Wq, np.float32)[:, ds]).astype(ndt),
            "wk": np.ascontiguousarray(
                np.asarray(Wk, np.float32)[:, ds]).astype(ndt),
            "wv": np.ascontiguousarray(
                np.asarray(Wv, np.float32)[:, ds]).astype(ndt),
            "wo": np.ascontiguousarray(np.asarray(Wo, np.float32)[ds, :]),
            "bq": np.asarray(bq, np.float32)[ds].reshape(DG, 1).copy(),
            "bk": np.asarray(bk, np.float32)[ds].reshape(DG, 1).copy(),
        })
    return in_maps


def kernel(x, Wq, bq, Wk, bk, Wv, bv, Wo, bo):
    mm_dt = _CACHE.get("mm_dt", "f16in")
    _CACHE["mm_dt"] = mm_dt
    if "nc" not in _CACHE:
        _CACHE["nc"] = build_kernel(mm_dt)
    nc = _CACHE["nc"]
    in_maps = _shard_inputs(x, Wq, bq, Wk, bk, Wv, bv, Wo, bo)
    res = run_bass_kernel_spmd(
        nc, in_maps, core_ids=list(range(N_CORES)), trace=False)
    out = np.zeros((B, S, D), np.float32)
    for core in range(N_CORES):
        out[core // 4] += res.results[core]["o"]
    # exact bias folding: +bo, + bv @ Wo (constant row vector)
    out += (np.asarray(bo, np.float32)
            + np.asarray(bv, np.float32) @ np.asarray(Wo, np.float32))
    return out



# revision 4
# speedup vs baseline: 1.1577x; 1.1577x over previous
"""Causal multi-head attention on 8 trn2 NeuronCores.

Problem: B=2, S=2048, D=1024, H=16 heads, HD=64. fp32 in/out.

Sharding: 8 cores = 2 (batch) x 4 (head groups of 4 heads).
Each core computes, for its batch b and head group g:
  Q^T,K^T  [256, 2048]  (d on partitions, seq on free)  = W^T-slice x
  V        [2048, 256+ones]  (natural, with a ones column per head)
  per 512-wide q chunk, per head:  S^T[k,q] = K^T.T @ Q^T  (PE, contraction 64),
  P~ = exp(S^T/8) (ACT), causal via block skipping + one merged gpsimd
  affine_select per diagonal 2-block group, PV: ctx_raw^T[d,q] accumulated
  over k tiles with V_aug stationary (m=65; row 64 = softmax denominator).
  Normalize: ACT copies pv psum -> sbuf (releases the psum bank early),
  DVE reciprocal_approx_fast on the den row, gpsimd partition_broadcast,
  DVE mul -> ctx^T.  Then per chunk O_partial = ctx^T.T @ Wo_rows.
Host: sums the 4 head-group partials per batch and adds bo + bv @ Wo
(exact: the bv bias contributes the constant row vector bv @ Wo_g).

Default mode "f16": ships x/Wq/Wk/Wv/Wo as fp16 and runs ALL matmuls in
native fp16 (1 cycle/row at any N, FWL fast weight loads) accumulating in
fp32 PSUM; p_t/qt/kt/vaug/ctxT/out in fp16 sbuf (halves DMA + sbuf
traffic).  Normalization internals (den, reciprocal, broadcast) in fp32.

"f16in" mode is the older float32r variant kept for comparison.
"""

import math
import sys

if "/opt/trn_rl_repo" not in sys.path:
    sys.path.insert(0, "/opt/trn_rl_repo")

import numpy as np

import concourse.bacc as bacc
import concourse.bass as bass
import concourse.mybir as mybir
import concourse.tile as tile
from concourse.bass_utils import run_bass_kernel_spmd

B, S, D, H = 2, 2048, 1024, 16
HD = D // H  # 64
N_CORES = 8
HEADS_PER_CORE = H // 4  # 4
DG = HEADS_PER_CORE * HD  # 256 head dims per core
P = 128
CHUNK = 512  # q chunk width
N_KT = S // P  # 16 k tiles
N_CH = S // CHUNK  # 4 q chunks
F32 = mybir.dt.float32
F16 = mybir.dt.float16

_CACHE = {}


def _mm(dt_name):
    return {"f16": F16,
            "f32r": mybir.dt.float32r, "f16in": mybir.dt.float32r,
            "f32": mybir.dt.float32,
            "bf16": mybir.dt.bfloat16}[dt_name]


def _in_dt(dt_name):
    """dtype for the x / Wq / Wk / Wv inputs (DMA-traffic dominant)."""
    return F16 if dt_name in ("f16in", "f16") else _mm(dt_name)


def build_kernel(mm_dt="f16", unroll=1, ablate=()):
    """Build + compile the per-core SPMD program. unroll>1 wraps the body
    in a hardware loop (for pure device timing measurements)."""
    mdt = _mm(mm_dt)
    idt = _in_dt(mm_dt)
    odt = F16 if mm_dt == "f16" else F32

    nc = bacc.Bacc("TRN2", target_bir_lowering=False, debug=False)
    xT_d = nc.dram_tensor("xT", [D, S], idt, kind="ExternalInput")
    wq_d = nc.dram_tensor("wq", [D, DG], idt, kind="ExternalInput")
    wk_d = nc.dram_tensor("wk", [D, DG], idt, kind="ExternalInput")
    wv_d = nc.dram_tensor("wv", [D, DG], idt, kind="ExternalInput")
    wo_d = nc.dram_tensor("wo", [DG, D], mdt, kind="ExternalInput")
    bq_d = nc.dram_tensor("bq", [DG, 1], F32, kind="ExternalInput")
    bk_d = nc.dram_tensor("bk", [DG, 1], F32, kind="ExternalInput")
    o_d = nc.dram_tensor("o", [S, D], odt, kind="ExternalOutput")

    NDT = D // P  # 8 contraction tiles over D
    NMT = DG // P  # 2 m-tiles over the core's head dims (= head pairs)

    with tile.TileContext(nc) as tc:
        def body(_iv=None):
            _body(tc, nc, mdt, idt, odt,
                  xT_d, wq_d, wk_d, wv_d, wo_d, bq_d, bk_d, o_d, NDT, NMT,
                  ablate)

        if unroll > 1:
            with tc.For_i(0, unroll, 1):
                body()
        else:
            body()

    nc.compile()
    return nc


def _body(tc, nc, mdt, idt, odt, xT_d, wq_d, wk_d, wv_d, wo_d, bq_d, bk_d,
          o_d, NDT, NMT, ablate=()):
    import contextlib
    ctx = contextlib.ExitStack()
    with ctx:
        const = ctx.enter_context(tc.tile_pool(name="const", bufs=1))
        sbuf = ctx.enter_context(tc.tile_pool(name="sbuf", bufs=1))
        ptile_p = ctx.enter_context(tc.tile_pool(name="ptile", bufs=8))
        craw_p = ctx.enter_context(tc.tile_pool(name="craw", bufs=2))
        rec_p = ctx.enter_context(tc.tile_pool(name="rec", bufs=2))
        recb_p = ctx.enter_context(tc.tile_pool(name="recb", bufs=2))
        out_p = ctx.enter_context(tc.tile_pool(name="outp", bufs=3))
        qkv_ps = ctx.enter_context(
            tc.tile_pool(name="qkv_ps", bufs=2, space="PSUM"))
        stp_ps = ctx.enter_context(
            tc.tile_pool(name="stp_ps", bufs=2, space="PSUM"))
        pv_ps = ctx.enter_context(
            tc.tile_pool(name="pv_ps", bufs=2, space="PSUM"))

        # ---- load inputs ------------------------------------------------
        # weights/biases first, then xt in chunk-major slices so chunk-0
        # compute starts early; all loads are queued before any output
        # stores (the HWDGE queue is in-order)
        xt = []
        for i in range(NDT):
            t = const.tile([P, S], idt, tag=f"xt{i}", name=f"xt{i}")
            xt.append(t)
        ws = {}
        for name, d in (("wq", wq_d), ("wk", wk_d), ("wv", wv_d)):
            ws[name] = [const.tile([P, DG], idt, tag=f"{name}{i}",
                                   name=f"{name}{i}") for i in range(NDT)]
        wo = [const.tile([P, D], mdt, tag=f"wo{m}", name=f"wo{m}")
              for m in range(NMT)]
        biases = {(name, m): const.tile([P, 1], F32, tag=f"{name}{m}",
                                        name=f"{name}{m}")
                  for name in ("bq", "bk") for m in range(NMT)}

        def dma_w(name, d):
            for i in range(NDT):
                nc.sync.dma_start(ws[name][i][:],
                                  d.ap()[P * i:P * (i + 1), :])

        def dma_xt(ci):
            csl = slice(CHUNK * ci, CHUNK * (ci + 1))
            for k in range(NDT):
                nc.sync.dma_start(xt[k][:, csl],
                                  xT_d.ap()[P * k:P * (k + 1), csl])

        # order: V(0)+QK(0) deps first, then remaining chunks, wo last
        dma_w("wv", wv_d)
        dma_xt(0)
        dma_w("wq", wq_d)
        dma_w("wk", wk_d)
        for (name, m), t in biases.items():
            d = bq_d if name == "bq" else bk_d
            nc.sync.dma_start(t[:], d.ap()[P * m:P * (m + 1), :])
        for ci in range(1, N_CH):
            dma_xt(ci)
        for m in range(NMT):
            nc.sync.dma_start(wo[m][:], wo_d.ap()[P * m:P * (m + 1), :])

        # ---- V projection (natural layout + ones cols) ------------------
        # vaug[j]: [128, 4*65]; per head h cols h*65..h*65+63 = V, col h*65+64 = 1
        ones_r = const.tile([P, HEADS_PER_CORE], mdt, tag="ones_r",
                            name="ones_r")
        nc.vector.memset(ones_r[:], 1.0)
        vaug = []
        for j in range(N_KT):
            t = sbuf.tile([P, HEADS_PER_CORE * (HD + 1)], mdt,
                          tag=f"vaug{j}", name=f"vaug{j}")
            vaug.append(t)

        def v_proj(j):
            ps = qkv_ps.tile([P, CHUNK], F32, tag="proj", name="proj")
            for k in range(NDT):
                nc.tensor.matmul(
                    ps[:, 0:DG],
                    xt[k][:, P * j:P * (j + 1)],
                    ws["wv"][k][:],
                    start=(k == 0), stop=(k == NDT - 1))
            dst = vaug[j][:].rearrange("p (h x) -> p h x", h=HEADS_PER_CORE)
            srcp = ps[:, 0:DG].rearrange("p (h x) -> p h x", h=HEADS_PER_CORE)
            nc.vector.tensor_copy(dst[:, :, 0:HD], srcp[:, :, :])
            nc.vector.tensor_copy(
                dst[:, :, HD:HD + 1],
                ones_r[:].rearrange("p (h x) -> p h x", x=1))

        # ---- Q^T / K^T projections (d on partitions) --------------------
        qt, kt = [], []
        for name, lst in (("wq", qt), ("wk", kt)):
            for m in range(NMT):
                t = sbuf.tile([P, S], mdt, tag=f"{name}T{m}",
                              name=f"{name}T{m}")
                lst.append(t)

        def qk_proj(ci):
            for name, lst in (("wq", qt), ("wk", kt)):
                bname = "bq" if name == "wq" else "bk"
                for m in range(NMT):
                    ps = qkv_ps.tile([P, CHUNK], F32, tag="proj", name="proj")
                    for k in range(NDT):
                        nc.tensor.matmul(
                            ps[:],
                            ws[name][k][:, P * m:P * (m + 1)],
                            xt[k][:, CHUNK * ci:CHUNK * (ci + 1)],
                            start=(k == 0), stop=(k == NDT - 1))
                    nc.vector.tensor_scalar_add(
                        lst[m][:, CHUNK * ci:CHUNK * (ci + 1)], ps[:],
                        biases[(bname, m)][:])

        # ---- attention + output projection, per q chunk -----------------
        ctxT = [sbuf.tile([P, S], mdt, tag=f"ctxT{m}", name=f"ctxT{m}")
                for m in range(NMT)]

        def wo_proj(ci):
            for qi in range(4):
                i = 4 * ci + qi
                ot = out_p.tile([P, D], odt, tag="ot", name="ot")
                for e in range(2):
                    ps = qkv_ps.tile([P, CHUNK], F32, tag="proj", name="proj")
                    for m in range(NMT):
                        nc.tensor.matmul(
                            ps[:],
                            ctxT[m][:, P * i:P * (i + 1)],
                            wo[m][:, CHUNK * e:CHUNK * (e + 1)],
                            start=(m == 0), stop=(m == NMT - 1))
                    nc.any.tensor_copy(ot[:, CHUNK * e:CHUNK * (e + 1)],
                                       ps[:])
                nc.sync.dma_start(o_d.ap()[P * i:P * (i + 1), :], ot[:])

        for ci in range(N_CH):
            for j in range(4 * ci, 4 * ci + 4):
                v_proj(j)
            qk_proj(ci)
            jmax = 4 * ci + 3  # last valid k tile for this chunk
            qsl = slice(CHUNK * ci, CHUNK * (ci + 1))
            for pair in range(NMT):
                pv = [pv_ps.tile([HD + 1, CHUNK], F32, tag="pv", name="pv")
                      for _ in range(2)]
                for j0 in range(0, jmax + 1, 2):
                    js = [j for j in (j0, j0 + 1) if j <= jmax]
                    nj = len(js)
                    pt = {}
                    for hh in range(2):  # head within pair
                        psl = slice(64 * hh, 64 * (hh + 1))
                        st = stp_ps.tile([P, 2 * CHUNK], F32, tag="stp",
                                         name="stp")
                        for gi, j in enumerate(js):
                            nc.tensor.matmul(
                                st[:, CHUNK * gi:CHUNK * (gi + 1)],
                                kt[pair][psl, P * j:P * (j + 1)],
                                qt[pair][psl, qsl],
                                start=True, stop=True)
                        p_t = ptile_p.tile([P, 2 * CHUNK], mdt, tag="ptile",
                                           name="ptile")
                        if "exp" in ablate:
                            nc.vector.tensor_copy(
                                p_t[:, 0:CHUNK * nj], st[:, 0:CHUNK * nj])
                        else:
                            nc.scalar.activation(
                                p_t[:, 0:CHUNK * nj], st[:, 0:CHUNK * nj],
                                mybir.ActivationFunctionType.Exp,
                                scale=0.125)
                        if j0 >= 4 * ci and "mask" not in ablate:
                            # both js are diagonal blocks: one merged
                            # affine_select; keep where q >= k + 128*dd
                            dd0 = j0 - 4 * ci
                            sel = p_t[:, 0:CHUNK * nj].rearrange(
                                "p (b q) -> p b q", b=nj)
                            nc.gpsimd.affine_select(
                                out=sel,
                                in_=sel,
                                compare_op=mybir.AluOpType.is_ge,
                                fill=0.0, base=-P * dd0,
                                pattern=[[-P, nj], [1, CHUNK]],
                                channel_multiplier=-1)
                        pt[hh] = p_t
                    for gi, j in enumerate(js):
                        for hh in range(2):
                            h = 2 * pair + hh
                            nc.tensor.matmul(
                                pv[hh][:],
                                vaug[j][:, (HD + 1) * h:(HD + 1) * (h + 1)],
                                pt[hh][:, CHUNK * gi:CHUNK * (gi + 1)],
                                start=(j == 0), stop=(j == jmax))
                # ---- normalize: copy psum out early (releases pv bank),
                # fast reciprocal of the den row, broadcast, multiply.
                if "div" in ablate:
                    for hh in range(2):
                        nc.vector.tensor_copy(
                            ctxT[pair][64 * hh:64 * (hh + 1), qsl],
                            pv[hh][0:HD, :])
                else:
                    for hh in range(2):
                        craw = craw_p.tile([HD, CHUNK], F32,
                                           tag=f"craw{hh}", name=f"craw{hh}")
                        nc.scalar.copy(craw[:], pv[hh][0:HD, :])
                        den = rec_p.tile([1, CHUNK], F32, tag=f"den{hh}",
                                         name=f"den{hh}")
                        # partition-base-shifting copy (64 -> 0); custom DVE
                        # ops need partition-0-based operands
                        nc.vector.tensor_copy(den[0:1, :],
                                              pv[hh][HD:HD + 1, :])
                        rec = rec_p.tile([1, CHUNK], F32, tag=f"rec{hh}",
                                         name=f"rec{hh}")
                        if "slowrecip" in ablate:
                            nc.vector.reciprocal(rec[0:1, :], den[0:1, :])
                        else:
                            nc.vector.reciprocal_approx_fast(
                                rec[0:1, :], den[0:1, :])
                        recb = recb_p.tile([HD, CHUNK], F32,
                                           tag=f"recb{hh}", name=f"recb{hh}")
                        nc.gpsimd.partition_broadcast(
                            recb[0:HD, :], rec[0:1, :])
                        nc.vector.tensor_mul(
                            ctxT[pair][64 * hh:64 * (hh + 1), qsl],
                            craw[0:HD, :],
                            recb[0:HD, :])
            wo_proj(ci)


def _shard_inputs(x, Wq, bq, Wk, bk, Wv, bv, Wo, bo):
    mm_dt = _CACHE.get("mm_dt", "f16")
    ndt = np.float16 if mm_dt in ("f16in", "f16") else np.float32
    wdt = np.float16 if mm_dt == "f16" else np.float32
    x = np.asarray(x, np.float32)
    in_maps = []
    for core in range(N_CORES):
        b, g = divmod(core, 4)
        ds = slice(DG * g, DG * (g + 1))
        in_maps.append({
            "xT": np.ascontiguousarray(x[b].T).astype(ndt),
            "wq": np.ascontiguousarray(
                np.asarray(Wq, np.float32)[:, ds]).astype(ndt),
            "wk": np.ascontiguousarray(
                np.asarray(Wk, np.float32)[:, ds]).astype(ndt),
            "wv": np.ascontiguousarray(
                np.asarray(Wv, np.float32)[:, ds]).astype(ndt),
            "wo": np.ascontiguousarray(
                np.asarray(Wo, np.float32)[ds, :]).astype(wdt),
            "bq": np.asarray(bq, np.float32)[ds].reshape(DG, 1).copy(),
            "bk": np.asarray(bk, np.float32)[ds].reshape(DG, 1).copy(),
        })
    return in_maps


def kernel(x, Wq, bq, Wk, bk, Wv, bv, Wo, bo):
    mm_dt = _CACHE.get("mm_dt", "f16")
    _CACHE["mm_dt"] = mm_dt
    if "nc" not in _CACHE:
        _CACHE["nc"] = build_kernel(mm_dt)
    nc = _CACHE["nc"]
    in_maps = _shard_inputs(x, Wq, bq, Wk, bk, Wv, bv, Wo, bo)
    res = run_bass_kernel_spmd(
        nc, in_maps, core_ids=list(range(N_CORES)), trace=False)
    out = np.zeros((B, S, D), np.float32)
    for core in range(N_CORES):
        out[core // 4] += res.results[core]["o"]
    # exact bias folding: +bo, + bv @ Wo (constant row vector)
    out += (np.asarray(bo, np.float32)
            + np.asarray(bv, np.float32) @ np.asarray(Wo, np.float32))
    return out


# revision 5
# speedup vs baseline: 1.2418x; 1.0727x over previous
"""Causal multi-head attention on 8 trn2 NeuronCores.

Problem: B=2, S=2048, D=1024, H=16 heads, HD=64. fp32 in/out.

Sharding: 8 cores = 2 (batch) x 4 (head groups of 4 heads).
Each core computes, for its batch b and head group g:
  Q^T,K^T  [256, 2048]  (d on partitions, seq on free)  = W^T-slice x
  V        [2048, 256+ones]  (natural, with a ones column per head)
  per 512-wide q chunk, per head:  S^T[k,q] = K^T.T @ Q^T  (PE, contraction 64),
  P~ = exp(S^T/8) (ACT), causal via block skipping + one merged gpsimd
  affine_select per diagonal 2-block group, PV: ctx_raw^T[d,q] accumulated
  over k tiles with V_aug stationary (m=65; row 64 = softmax denominator).
  Normalize: DVE copies pv psum -> sbuf (releases the psum bank early),
  DVE reciprocal_approx_fast on the den row, gpsimd partition_broadcast,
  DVE mul -> ctx^T.  Then per chunk O_partial = ctx^T.T @ Wo_rows.
Host: sums the 4 head-group partials per batch and adds bo + bv @ Wo
(exact: the bv bias contributes the constant row vector bv @ Wo_g).

Emission order pipelines chunks: proj(ci) -> Wo(ci-1) -> attention(ci),
so the normalize chain of chunk ci-1 and Wo wait-time overlap the next
chunk's projections.  All input DMAs are single batched descriptors
(1 per weight tensor, 1 per x chunk); output stores are batched per
chunk and issued from the scalar-engine HWDGE queue so the sync queue
(input loads) is never blocked behind stores at the loop boundary.

Default mode "f16": ships x/Wq/Wk/Wv/Wo as fp16 and runs ALL matmuls in
native fp16 (1 cycle/row at any N, FWL fast weight loads) accumulating in
fp32 PSUM; p_t/qt/kt/vaug/ctxT/out in fp16 sbuf.  Normalization
internals (den, reciprocal, broadcast) in fp32.
"""

import math
import sys

if "/opt/trn_rl_repo" not in sys.path:
    sys.path.insert(0, "/opt/trn_rl_repo")

import numpy as np

import concourse.bacc as bacc
import concourse.bass as bass
import concourse.mybir as mybir
import concourse.tile as tile
from concourse.bass_utils import run_bass_kernel_spmd

B, S, D, H = 2, 2048, 1024, 16
HD = D // H  # 64
N_CORES = 8
HEADS_PER_CORE = H // 4  # 4
DG = HEADS_PER_CORE * HD  # 256 head dims per core
P = 128
CHUNK = 512  # q chunk width
N_KT = S // P  # 16 k tiles
N_CH = S // CHUNK  # 4 q chunks
F32 = mybir.dt.float32
F16 = mybir.dt.float16

_CACHE = {}


def _mm(dt_name):
    return {"f16": F16,
            "f32r": mybir.dt.float32r, "f16in": mybir.dt.float32r,
            "f32": mybir.dt.float32,
            "bf16": mybir.dt.bfloat16}[dt_name]


def _in_dt(dt_name):
    """dtype for the x / Wq / Wk / Wv inputs (DMA-traffic dominant)."""
    return F16 if dt_name in ("f16in", "f16") else _mm(dt_name)


def build_kernel(mm_dt="f16", unroll=1, ablate=()):
    """Build + compile the per-core SPMD program. unroll>1 wraps the body
    in a hardware loop (for pure device timing measurements)."""
    mdt = _mm(mm_dt)
    idt = _in_dt(mm_dt)
    odt = F16 if mm_dt == "f16" else F32

    nc = bacc.Bacc("TRN2", target_bir_lowering=False, debug=False)
    xT_d = nc.dram_tensor("xT", [D, S], idt, kind="ExternalInput")
    wq_d = nc.dram_tensor("wq", [D, DG], idt, kind="ExternalInput")
    wk_d = nc.dram_tensor("wk", [D, DG], idt, kind="ExternalInput")
    wv_d = nc.dram_tensor("wv", [D, DG], idt, kind="ExternalInput")
    wo_d = nc.dram_tensor("wo", [DG, D], mdt, kind="ExternalInput")
    bq_d = nc.dram_tensor("bq", [DG, 1], F32, kind="ExternalInput")
    bk_d = nc.dram_tensor("bk", [DG, 1], F32, kind="ExternalInput")
    o_d = nc.dram_tensor("o", [S, D], odt, kind="ExternalOutput")

    NDT = D // P  # 8 contraction tiles over D
    NMT = DG // P  # 2 m-tiles over the core's head dims (= head pairs)

    with tile.TileContext(nc) as tc:
        with tc.tile_pool(name="warm", bufs=1) as warm_p:
            # tiny pre-loop exp so the ACT table set is loaded once in the
            # preamble, not on every loop iteration
            wt = warm_p.tile([1, 8], F32, tag="warm", name="warm")
            nc.vector.memset(wt[:], 0.0)
            nc.scalar.activation(wt[:], wt[:],
                                 mybir.ActivationFunctionType.Exp)

        def body(_iv=None):
            _body(tc, nc, mdt, idt, odt,
                  xT_d, wq_d, wk_d, wv_d, wo_d, bq_d, bk_d, o_d, NDT, NMT,
                  ablate)

        if unroll > 1:
            with tc.For_i(0, unroll, 1):
                body()
        else:
            body()

    nc.compile()
    return nc


def _body(tc, nc, mdt, idt, odt, xT_d, wq_d, wk_d, wv_d, wo_d, bq_d, bk_d,
          o_d, NDT, NMT, ablate=()):
    import contextlib
    ctx = contextlib.ExitStack()
    with ctx:
        const = ctx.enter_context(tc.tile_pool(name="const", bufs=1))
        sbuf = ctx.enter_context(tc.tile_pool(name="sbuf", bufs=1))
        ptile_p = ctx.enter_context(tc.tile_pool(name="ptile", bufs=8))
        craw_p = ctx.enter_context(tc.tile_pool(name="craw", bufs=2))
        rec_p = ctx.enter_context(tc.tile_pool(name="rec", bufs=2))
        recb_p = ctx.enter_context(tc.tile_pool(name="recb", bufs=2))
        out_p = ctx.enter_context(tc.tile_pool(name="outp", bufs=2))
        qkv_ps = ctx.enter_context(
            tc.tile_pool(name="qkv_ps", bufs=2, space="PSUM"))
        stp_ps = ctx.enter_context(
            tc.tile_pool(name="stp_ps", bufs=2, space="PSUM"))
        pv_ps = ctx.enter_context(
            tc.tile_pool(name="pv_ps", bufs=2, space="PSUM"))

        # ---- load inputs (batched descriptors, sync queue = loads only) --
        xt = const.tile([P, NDT * S], idt, tag="xt", name="xt")
        ws = {name: const.tile([P, NDT * DG], idt, tag=name, name=name)
              for name in ("wq", "wk", "wv")}
        wo = const.tile([P, NMT * D], mdt, tag="wo", name="wo")
        biases = {(name, m): const.tile([P, 1], F32, tag=f"{name}{m}",
                                        name=f"{name}{m}")
                  for name in ("bq", "bk") for m in range(NMT)}

        def xts(k, cols):
            """xt slice for contraction tile k."""
            return xt[:, S * k + cols.start:S * k + cols.stop]

        def wss(name, k, c0=0, c1=DG):
            return ws[name][:, DG * k + c0:DG * k + c1]

        def dma_w(name, d):
            dst = ws[name][:].rearrange("p (k c) -> p k c", k=NDT)
            src = d.ap().rearrange("(k p) c -> p k c", k=NDT)
            nc.sync.dma_start(dst, src)

        def dma_xt(ci):
            csl = slice(CHUNK * ci, CHUNK * (ci + 1))
            dst = xt[:].rearrange("p (k s) -> p k s", k=NDT)[:, :, csl]
            src = xT_d.ap().rearrange("(k p) s -> p k s", k=NDT)[:, :, csl]
            nc.sync.dma_start(dst, src)

        # order: V(0)+QK(0) deps first, then remaining chunks, wo last
        dma_w("wv", wv_d)
        dma_xt(0)
        dma_w("wq", wq_d)
        dma_w("wk", wk_d)
        for (name, m), t in biases.items():
            d = bq_d if name == "bq" else bk_d
            nc.sync.dma_start(t[:], d.ap()[P * m:P * (m + 1), :])
        for ci in range(1, N_CH):
            dma_xt(ci)
        nc.sync.dma_start(wo[:].rearrange("p (m d) -> p m d", m=NMT),
                          wo_d.ap().rearrange("(m p) d -> p m d", m=NMT))

        # ---- V projection (natural layout + ones cols) ------------------
        # vaug[j]: [128, 4*65]; per head h cols h*65..h*65+63 = V, col h*65+64 = 1
        ones_r = const.tile([P, HEADS_PER_CORE], mdt, tag="ones_r",
                            name="ones_r")
        nc.vector.memset(ones_r[:], 1.0)
        vaug = []
        for j in range(N_KT):
            t = sbuf.tile([P, HEADS_PER_CORE * (HD + 1)], mdt,
                          tag=f"vaug{j}", name=f"vaug{j}")
            vaug.append(t)

        def v_proj(j):
            ps = qkv_ps.tile([P, CHUNK], F32, tag="proj", name="proj")
            for k in range(NDT):
                nc.tensor.matmul(
                    ps[:, 0:DG],
                    xts(k, slice(P * j, P * (j + 1))),
                    wss("wv", k),
                    start=(k == 0), stop=(k == NDT - 1))
            dst = vaug[j][:].rearrange("p (h x) -> p h x", h=HEADS_PER_CORE)
            srcp = ps[:, 0:DG].rearrange("p (h x) -> p h x", h=HEADS_PER_CORE)
            nc.vector.tensor_copy(dst[:, :, 0:HD], srcp[:, :, :])
            nc.vector.tensor_copy(
                dst[:, :, HD:HD + 1],
                ones_r[:].rearrange("p (h x) -> p h x", x=1))

        # ---- Q^T / K^T projections (d on partitions) --------------------
        qt, kt = [], []
        for name, lst in (("wq", qt), ("wk", kt)):
            for m in range(NMT):
                t = sbuf.tile([P, S], mdt, tag=f"{name}T{m}",
                              name=f"{name}T{m}")
                lst.append(t)

        def qk_proj(ci):
            for name, lst in (("wq", qt), ("wk", kt)):
                bname = "bq" if name == "wq" else "bk"
                for m in range(NMT):
                    ps = qkv_ps.tile([P, CHUNK], F32, tag="proj", name="proj")
                    for k in range(NDT):
                        nc.tensor.matmul(
                            ps[:],
                            wss(name, k, P * m, P * (m + 1)),
                            xts(k, slice(CHUNK * ci, CHUNK * (ci + 1))),
                            start=(k == 0), stop=(k == NDT - 1))
                    nc.vector.tensor_scalar_add(
                        lst[m][:, CHUNK * ci:CHUNK * (ci + 1)], ps[:],
                        biases[(bname, m)][:])

        # ---- attention + output projection, per q chunk -----------------
        ctxT = [sbuf.tile([P, S], mdt, tag=f"ctxT{m}", name=f"ctxT{m}")
                for m in range(NMT)]

        def wo_proj(ci):
            ot = out_p.tile([P, 4 * D], odt, tag="ot", name="ot")
            for qi in range(4):
                i = 4 * ci + qi
                for e in range(2):
                    ps = qkv_ps.tile([P, CHUNK], F32, tag="proj", name="proj")
                    for m in range(NMT):
                        nc.tensor.matmul(
                            ps[:],
                            ctxT[m][:, P * i:P * (i + 1)],
                            wo[:, D * m + CHUNK * e:D * m + CHUNK * (e + 1)],
                            start=(m == 0), stop=(m == NMT - 1))
                    nc.any.tensor_copy(
                        ot[:, D * qi + CHUNK * e:D * qi + CHUNK * (e + 1)],
                        ps[:])
            # batched store for the whole chunk, issued from the scalar
            # (ACT) HWDGE queue: keeps the sync queue free for loads
            dst = o_d.ap()[CHUNK * ci:CHUNK * (ci + 1), :].rearrange(
                "(q p) d -> p q d", q=4)
            nc.scalar.dma_start(dst, ot[:].rearrange("p (q d) -> p q d", q=4))

        def attention(ci):
            jmax = 4 * ci + 3  # last valid k tile for this chunk
            qsl = slice(CHUNK * ci, CHUNK * (ci + 1))
            for pair in range(NMT):
                pv = [pv_ps.tile([HD + 1, CHUNK], F32, tag="pv", name="pv")
                      for _ in range(2)]
                for j0 in range(0, jmax + 1, 2):
                    js = [j for j in (j0, j0 + 1) if j <= jmax]
                    nj = len(js)
                    pt = {}
                    for hh in range(2):  # head within pair
                        psl = slice(64 * hh, 64 * (hh + 1))
                        st = stp_ps.tile([P, 2 * CHUNK], F32, tag="stp",
                                         name="stp")
                        for gi, j in enumerate(js):
                            nc.tensor.matmul(
                                st[:, CHUNK * gi:CHUNK * (gi + 1)],
                                kt[pair][psl, P * j:P * (j + 1)],
                                qt[pair][psl, qsl],
                                start=True, stop=True)
                        p_t = ptile_p.tile([P, 2 * CHUNK], mdt, tag="ptile",
                                           name="ptile")
                        if "exp" in ablate:
                            nc.vector.tensor_copy(
                                p_t[:, 0:CHUNK * nj], st[:, 0:CHUNK * nj])
                        else:
                            nc.scalar.activation(
                                p_t[:, 0:CHUNK * nj], st[:, 0:CHUNK * nj],
                                mybir.ActivationFunctionType.Exp,
                                scale=0.125)
                        if j0 >= 4 * ci and "mask" not in ablate:
                            # both js are diagonal blocks: one merged
                            # affine_select; keep where q >= k + 128*dd
                            dd0 = j0 - 4 * ci
                            sel = p_t[:, 0:CHUNK * nj].rearrange(
                                "p (b q) -> p b q", b=nj)
                            nc.gpsimd.affine_select(
                                out=sel,
                                in_=sel,
                                compare_op=mybir.AluOpType.is_ge,
                                fill=0.0, base=-P * dd0,
                                pattern=[[-P, nj], [1, CHUNK]],
                                channel_multiplier=-1)
                        pt[hh] = p_t
                    for gi, j in enumerate(js):
                        for hh in range(2):
                            h = 2 * pair + hh
                            nc.tensor.matmul(
                                pv[hh][:],
                                vaug[j][:, (HD + 1) * h:(HD + 1) * (h + 1)],
                                pt[hh][:, CHUNK * gi:CHUNK * (gi + 1)],
                                start=(j == 0), stop=(j == jmax))
                # ---- normalize: copy psum out early (releases pv bank),
                # fast reciprocal of the den row, broadcast, multiply.
                if "div" in ablate:
                    for hh in range(2):
                        nc.vector.tensor_copy(
                            ctxT[pair][64 * hh:64 * (hh + 1), qsl],
                            pv[hh][0:HD, :])
                else:
                    for hh in range(2):
                        craw = craw_p.tile([HD, CHUNK], F32,
                                           tag=f"craw{hh}", name=f"craw{hh}")
                        nc.vector.tensor_copy(craw[:], pv[hh][0:HD, :])
                        den = rec_p.tile([1, CHUNK], F32, tag=f"den{hh}",
                                         name=f"den{hh}")
                        # partition-base-shifting copy (64 -> 0); custom DVE
                        # ops need partition-0-based operands
                        nc.vector.tensor_copy(den[0:1, :],
                                              pv[hh][HD:HD + 1, :])
                        rec = rec_p.tile([1, CHUNK], F32, tag=f"rec{hh}",
                                         name=f"rec{hh}")
                        nc.vector.reciprocal_approx_fast(
                            rec[0:1, :], den[0:1, :])
                        recb = recb_p.tile([HD, CHUNK], F32,
                                           tag=f"recb{hh}", name=f"recb{hh}")
                        nc.gpsimd.partition_broadcast(
                            recb[0:HD, :], rec[0:1, :])
                        nc.vector.tensor_mul(
                            ctxT[pair][64 * hh:64 * (hh + 1), qsl],
                            craw[0:HD, :],
                            recb[0:HD, :])

        # pipelined emission: proj(ci) -> Wo(ci-1) -> attention(ci)
        for ci in range(N_CH):
            for j in range(4 * ci, 4 * ci + 4):
                v_proj(j)
            qk_proj(ci)
            if ci > 0:
                wo_proj(ci - 1)
            attention(ci)
        wo_proj(N_CH - 1)


def _shard_inputs(x, Wq, bq, Wk, bk, Wv, bv, Wo, bo):
    mm_dt = _CACHE.get("mm_dt", "f16")
    ndt = np.float16 if mm_dt in ("f16in", "f16") else np.float32
    wdt = np.float16 if mm_dt == "f16" else np.float32
    x = np.asarray(x, np.float32)
    in_maps = []
    for core in range(N_CORES):
        b, g = divmod(core, 4)
        ds = slice(DG * g, DG * (g + 1))
        in_maps.append({
            "xT": np.ascontiguousarray(x[b].T).astype(ndt),
            "wq": np.ascontiguousarray(
                np.asarray(Wq, np.float32)[:, ds]).astype(ndt),
            "wk": np.ascontiguousarray(
                np.asarray(Wk, np.float32)[:, ds]).astype(ndt),
            "wv": np.ascontiguousarray(
                np.asarray(Wv, np.float32)[:, ds]).astype(ndt),
            "wo": np.ascontiguousarray(
                np.asarray(Wo, np.float32)[ds, :]).astype(wdt),
            "bq": np.asarray(bq, np.float32)[ds].reshape(DG, 1).copy(),
            "bk": np.asarray(bk, np.float32)[ds].reshape(DG, 1).copy(),
        })
    return in_maps


def kernel(x, Wq, bq, Wk, bk, Wv, bv, Wo, bo):
    mm_dt = _CACHE.get("mm_dt", "f16")
    _CACHE["mm_dt"] = mm_dt
    if "nc" not in _CACHE:
        _CACHE["nc"] = build_kernel(mm_dt)
    nc = _CACHE["nc"]
    in_maps = _shard_inputs(x, Wq, bq, Wk, bk, Wv, bv, Wo, bo)
    res = run_bass_kernel_spmd(
        nc, in_maps, core_ids=list(range(N_CORES)), trace=False)
    out = np.zeros((B, S, D), np.float32)
    for core in range(N_CORES):
        out[core // 4] += res.results[core]["o"]
    # exact bias folding: +bo, + bv @ Wo (constant row vector)
    out += (np.asarray(bo, np.float32)
            + np.asarray(bv, np.float32) @ np.asarray(Wo, np.float32))
    return out


# revision 7
# speedup vs baseline: 1.2582x; 1.0132x over previous
"""Causal multi-head attention on 8 trn2 NeuronCores.

Problem: B=2, S=2048, D=1024, H=16 heads, HD=64. fp32 in/out.

Sharding: 8 cores = 2 (batch) x 4 (head groups of 4 heads).
Each core computes, for its batch b and head group g:
  Q^T,K^T  [256, 2048]  (d on partitions, seq on free)  = W^T-slice x
  V        [2048, 256+ones]  (natural, with a ones column per head)
  per 512-wide q chunk, per head:  S^T[k,q] = K^T.T @ Q^T  (PE, contraction 64),
  P~ = exp(S^T/8) (ACT), causal via block skipping + one merged gpsimd
  affine_select per diagonal 2-block group, PV: ctx_raw^T[d,q] accumulated
  over k tiles with V_aug stationary (m=65; row 64 = softmax denominator).
  Normalize: DVE copies pv psum -> sbuf (releases the psum bank early),
  DVE reciprocal_approx_fast on the den row, gpsimd partition_broadcast,
  DVE mul -> ctx^T.  Then per chunk O_partial = ctx^T.T @ Wo_rows.
Host: sums the 4 head-group partials per batch and adds bo + bv @ Wo
(exact: the bv bias contributes the constant row vector bv @ Wo_g).

Emission order pipelines chunks: proj(ci) -> Wo(ci-1) -> attention(ci),
so the normalize chain of chunk ci-1 and Wo wait-time overlap the next
chunk's projections.  All input DMAs are single batched descriptors
(1 per weight tensor, 1 per x chunk); output stores are batched per
chunk and issued from the scalar-engine HWDGE queue so the sync queue
(input loads) is never blocked behind stores at the loop boundary.

Default mode "f16": ships x/Wq/Wk/Wv/Wo as fp16 and runs ALL matmuls in
native fp16 (1 cycle/row at any N, FWL fast weight loads) accumulating in
fp32 PSUM; p_t/qt/kt/vaug/ctxT/out in fp16 sbuf.  Normalization
internals (den, reciprocal, broadcast) in fp32.
"""

import math
import sys

if "/opt/trn_rl_repo" not in sys.path:
    sys.path.insert(0, "/opt/trn_rl_repo")

import numpy as np

import concourse.bacc as bacc
import concourse.bass as bass
import concourse.mybir as mybir
import concourse.tile as tile
from concourse.bass_utils import run_bass_kernel_spmd

B, S, D, H = 2, 2048, 1024, 16
HD = D // H  # 64
N_CORES = 8
HEADS_PER_CORE = H // 4  # 4
DG = HEADS_PER_CORE * HD  # 256 head dims per core
P = 128
CHUNK = 512  # q chunk width
N_KT = S // P  # 16 k tiles
N_CH = S // CHUNK  # 4 q chunks
F32 = mybir.dt.float32
F16 = mybir.dt.float16

_CACHE = {}


def _mm(dt_name):
    return {"f16": F16,
            "f32r": mybir.dt.float32r, "f16in": mybir.dt.float32r,
            "f32": mybir.dt.float32,
            "bf16": mybir.dt.bfloat16}[dt_name]


def _in_dt(dt_name):
    """dtype for the x / Wq / Wk / Wv inputs (DMA-traffic dominant)."""
    return F16 if dt_name in ("f16in", "f16") else _mm(dt_name)


def build_kernel(mm_dt="f16", unroll=1, ablate=()):
    """Build + compile the per-core SPMD program. unroll>1 wraps the body
    in a hardware loop (for pure device timing measurements)."""
    mdt = _mm(mm_dt)
    idt = _in_dt(mm_dt)
    odt = F16 if mm_dt == "f16" else F32

    nc = bacc.Bacc("TRN2", target_bir_lowering=False, debug=False)
    xT_d = nc.dram_tensor("xT", [D, S], idt, kind="ExternalInput")
    wq_d = nc.dram_tensor("wq", [D, DG], idt, kind="ExternalInput")
    wk_d = nc.dram_tensor("wk", [D, DG], idt, kind="ExternalInput")
    wv_d = nc.dram_tensor("wv", [D, DG], idt, kind="ExternalInput")
    wo_d = nc.dram_tensor("wo", [DG, D], mdt, kind="ExternalInput")
    # packed per-partition biases: col 0,1 = bq m-tiles, col 2,3 = bk m-tiles
    bias_d = nc.dram_tensor("bias", [P, 4], F32, kind="ExternalInput")
    o_d = nc.dram_tensor("o", [S, D], odt, kind="ExternalOutput")

    NDT = D // P  # 8 contraction tiles over D
    NMT = DG // P  # 2 m-tiles over the core's head dims (= head pairs)

    with tile.TileContext(nc) as tc:
        with tc.tile_pool(name="warm", bufs=1) as warm_p:
            # tiny pre-loop exp so the ACT table set is loaded once in the
            # preamble, not on every loop iteration
            wt = warm_p.tile([1, 8], F32, tag="warm", name="warm")
            nc.vector.memset(wt[:], 0.0)
            nc.scalar.activation(wt[:], wt[:],
                                 mybir.ActivationFunctionType.Exp)

        def body(_iv=None):
            _body(tc, nc, mdt, idt, odt,
                  xT_d, wq_d, wk_d, wv_d, wo_d, bias_d, o_d, NDT, NMT,
                  ablate)

        if unroll > 1:
            with tc.For_i(0, unroll, 1):
                body()
        else:
            body()

    nc.compile()
    return nc


def _body(tc, nc, mdt, idt, odt, xT_d, wq_d, wk_d, wv_d, wo_d, bias_d,
          o_d, NDT, NMT, ablate=()):
    import contextlib
    ctx = contextlib.ExitStack()
    with ctx:
        const = ctx.enter_context(tc.tile_pool(name="const", bufs=1))
        sbuf = ctx.enter_context(tc.tile_pool(name="sbuf", bufs=1))
        ptile_p = ctx.enter_context(tc.tile_pool(name="ptile", bufs=8))
        craw_p = ctx.enter_context(tc.tile_pool(name="craw", bufs=2))
        rec_p = ctx.enter_context(tc.tile_pool(name="rec", bufs=2))
        recb_p = ctx.enter_context(tc.tile_pool(name="recb", bufs=2))
        out_p = ctx.enter_context(tc.tile_pool(name="outp", bufs=2))
        qkv_ps = ctx.enter_context(
            tc.tile_pool(name="qkv_ps", bufs=2, space="PSUM"))
        stp_ps = ctx.enter_context(
            tc.tile_pool(name="stp_ps", bufs=2, space="PSUM"))
        pv_ps = ctx.enter_context(
            tc.tile_pool(name="pv_ps", bufs=2, space="PSUM"))

        # ---- load inputs (batched descriptors, sync queue = loads only) --
        xt = const.tile([P, NDT * S], idt, tag="xt", name="xt")
        ws = {name: const.tile([P, NDT * DG], idt, tag=name, name=name)
              for name in ("wq", "wk", "wv")}
        wo = const.tile([P, NMT * D], mdt, tag="wo", name="wo")
        bias_t = const.tile([P, 4], F32, tag="bias", name="bias")
        biases = {("bq", 0): bias_t[:, 0:1], ("bq", 1): bias_t[:, 1:2],
                  ("bk", 0): bias_t[:, 2:3], ("bk", 1): bias_t[:, 3:4]}

        def xts(k, cols):
            """xt slice for contraction tile k."""
            return xt[:, S * k + cols.start:S * k + cols.stop]

        def wss(name, k, c0=0, c1=DG):
            return ws[name][:, DG * k + c0:DG * k + c1]

        def dma_w(name, d):
            dst = ws[name][:].rearrange("p (k c) -> p k c", k=NDT)
            src = d.ap().rearrange("(k p) c -> p k c", k=NDT)
            nc.sync.dma_start(dst, src)

        def dma_xt(ci):
            csl = slice(CHUNK * ci, CHUNK * (ci + 1))
            dst = xt[:].rearrange("p (k s) -> p k s", k=NDT)[:, :, csl]
            src = xT_d.ap().rearrange("(k p) s -> p k s", k=NDT)[:, :, csl]
            nc.sync.dma_start(dst, src)

        # order: V(0)+QK(0) deps first, then remaining chunks, wo last
        dma_w("wv", wv_d)
        dma_xt(0)
        dma_w("wq", wq_d)
        dma_w("wk", wk_d)
        nc.sync.dma_start(bias_t[:], bias_d.ap()[:, :])
        for ci in range(1, N_CH):
            dma_xt(ci)
        nc.sync.dma_start(wo[:].rearrange("p (m d) -> p m d", m=NMT),
                          wo_d.ap().rearrange("(m p) d -> p m d", m=NMT))

        # ---- V projection (natural layout + ones cols) ------------------
        # vaug[j]: [128, 4*65]; per head h cols h*65..h*65+63 = V, col h*65+64 = 1
        ones_r = const.tile([P, HEADS_PER_CORE], mdt, tag="ones_r",
                            name="ones_r")
        nc.vector.memset(ones_r[:], 1.0)
        vaug = []
        for j in range(N_KT):
            t = sbuf.tile([P, HEADS_PER_CORE * (HD + 1)], mdt,
                          tag=f"vaug{j}", name=f"vaug{j}")
            vaug.append(t)

        def v_proj(j):
            ps = qkv_ps.tile([P, CHUNK], F32, tag="proj", name="proj")
            for k in range(NDT):
                nc.tensor.matmul(
                    ps[:, 0:DG],
                    xts(k, slice(P * j, P * (j + 1))),
                    wss("wv", k),
                    start=(k == 0), stop=(k == NDT - 1))
            dst = vaug[j][:].rearrange("p (h x) -> p h x", h=HEADS_PER_CORE)
            srcp = ps[:, 0:DG].rearrange("p (h x) -> p h x", h=HEADS_PER_CORE)
            nc.vector.tensor_copy(dst[:, :, 0:HD], srcp[:, :, :])
            nc.vector.tensor_copy(
                dst[:, :, HD:HD + 1],
                ones_r[:].rearrange("p (h x) -> p h x", x=1))

        # ---- Q^T / K^T projections (d on partitions) --------------------
        qt, kt = [], []
        for name, lst in (("wq", qt), ("wk", kt)):
            for m in range(NMT):
                t = sbuf.tile([P, S], mdt, tag=f"{name}T{m}",
                              name=f"{name}T{m}")
                lst.append(t)

        def qk_proj(ci):
            for name, lst in (("wq", qt), ("wk", kt)):
                bname = "bq" if name == "wq" else "bk"
                for m in range(NMT):
                    ps = qkv_ps.tile([P, CHUNK], F32, tag="proj", name="proj")
                    for k in range(NDT):
                        nc.tensor.matmul(
                            ps[:],
                            wss(name, k, P * m, P * (m + 1)),
                            xts(k, slice(CHUNK * ci, CHUNK * (ci + 1))),
                            start=(k == 0), stop=(k == NDT - 1))
                    nc.vector.tensor_scalar_add(
                        lst[m][:, CHUNK * ci:CHUNK * (ci + 1)], ps[:],
                        biases[(bname, m)])

        # ---- attention + output projection, per q chunk -----------------
        ctxT = [sbuf.tile([P, S], mdt, tag=f"ctxT{m}", name=f"ctxT{m}")
                for m in range(NMT)]

        def wo_proj(ci):
            ot = out_p.tile([P, 4 * D], odt, tag="ot", name="ot")
            for qi in range(4):
                i = 4 * ci + qi
                for e in range(2):
                    ps = qkv_ps.tile([P, CHUNK], F32, tag="proj", name="proj")
                    for m in range(NMT):
                        nc.tensor.matmul(
                            ps[:],
                            ctxT[m][:, P * i:P * (i + 1)],
                            wo[:, D * m + CHUNK * e:D * m + CHUNK * (e + 1)],
                            start=(m == 0), stop=(m == NMT - 1))
                    nc.any.tensor_copy(
                        ot[:, D * qi + CHUNK * e:D * qi + CHUNK * (e + 1)],
                        ps[:])
            # batched store for the whole chunk, issued from the scalar
            # (ACT) HWDGE queue: keeps the sync queue free for loads
            dst = o_d.ap()[CHUNK * ci:CHUNK * (ci + 1), :].rearrange(
                "(q p) d -> p q d", q=4)
            nc.scalar.dma_start(dst, ot[:].rearrange("p (q d) -> p q d", q=4))

        def attention(ci):
            jmax = 4 * ci + 3  # last valid k tile for this chunk
            qsl = slice(CHUNK * ci, CHUNK * (ci + 1))
            for pair in range(NMT):
                pv = [pv_ps.tile([HD + 1, CHUNK], F32, tag="pv", name="pv")
                      for _ in range(2)]
                for j0 in range(0, jmax + 1, 2):
                    js = [j for j in (j0, j0 + 1) if j <= jmax]
                    nj = len(js)
                    pt = {}
                    for hh in range(2):  # head within pair
                        psl = slice(64 * hh, 64 * (hh + 1))
                        st = stp_ps.tile([P, 2 * CHUNK], F32, tag="stp",
                                         name="stp")
                        for gi, j in enumerate(js):
                            nc.tensor.matmul(
                                st[:, CHUNK * gi:CHUNK * (gi + 1)],
                                kt[pair][psl, P * j:P * (j + 1)],
                                qt[pair][psl, qsl],
                                start=True, stop=True)
                        p_t = ptile_p.tile([P, 2 * CHUNK], mdt, tag="ptile",
                                           name="ptile")
                        if "exp" in ablate:
                            nc.vector.tensor_copy(
                                p_t[:, 0:CHUNK * nj], st[:, 0:CHUNK * nj])
                        else:
                            nc.scalar.activation(
                                p_t[:, 0:CHUNK * nj], st[:, 0:CHUNK * nj],
                                mybir.ActivationFunctionType.Exp,
                                scale=0.125)
                        if j0 >= 4 * ci and "mask" not in ablate:
                            # both js are diagonal blocks: one merged
                            # affine_select; keep where q >= k + 128*dd
                            dd0 = j0 - 4 * ci
                            sel = p_t[:, 0:CHUNK * nj].rearrange(
                                "p (b q) -> p b q", b=nj)
                            nc.gpsimd.affine_select(
                                out=sel,
                                in_=sel,
                                compare_op=mybir.AluOpType.is_ge,
                                fill=0.0, base=-P * dd0,
                                pattern=[[-P, nj], [1, CHUNK]],
                                channel_multiplier=-1)
                        pt[hh] = p_t
                    for gi, j in enumerate(js):
                        for hh in range(2):
                            h = 2 * pair + hh
                            nc.tensor.matmul(
                                pv[hh][:],
                                vaug[j][:, (HD + 1) * h:(HD + 1) * (h + 1)],
                                pt[hh][:, CHUNK * gi:CHUNK * (gi + 1)],
                                start=(j == 0), stop=(j == jmax))
                # ---- normalize phase 1: copy psum out (releases the pv
                # bank early).  Phase 2 (recip/broadcast/mul) is deferred
                # past the next chunk's projections so it never blocks the
                # qk evacuations in DVE program order.
                if "div" in ablate:
                    for hh in range(2):
                        nc.vector.tensor_copy(
                            ctxT[pair][64 * hh:64 * (hh + 1), qsl],
                            pv[hh][0:HD, :])
                else:
                    for hh in range(2):
                        craw = craw_p.tile([HD, CHUNK], F32,
                                           tag=f"craw{hh}", name=f"craw{hh}")
                        nc.vector.tensor_copy(craw[:], pv[hh][0:HD, :])
                        den = rec_p.tile([1, CHUNK], F32, tag=f"den{hh}",
                                         name=f"den{hh}")
                        # partition-base-shifting copy (64 -> 0); custom DVE
                        # ops need partition-0-based operands
                        nc.vector.tensor_copy(den[0:1, :],
                                              pv[hh][HD:HD + 1, :])
                        norm2_work.append((pair, hh, qsl, craw, den))

        def norm2_flush():
            while norm2_work:
                pair, hh, qsl, craw, den = norm2_work.pop(0)
                rec = rec_p.tile([1, CHUNK], F32, tag=f"rec{hh}",
                                 name=f"rec{hh}")
                nc.vector.reciprocal_approx_fast(rec[0:1, :], den[0:1, :])
                recb = recb_p.tile([HD, CHUNK], F32,
                                   tag=f"recb{hh}", name=f"recb{hh}")
                nc.gpsimd.partition_broadcast(recb[0:HD, :], rec[0:1, :])
                nc.vector.tensor_mul(
                    ctxT[pair][64 * hh:64 * (hh + 1), qsl],
                    craw[0:HD, :],
                    recb[0:HD, :])

        # pipelined emission: proj(ci) -> norm2+Wo(ci-1) -> attention(ci)
        norm2_work = []
        for ci in range(N_CH):
            for j in range(4 * ci, 4 * ci + 4):
                v_proj(j)
            qk_proj(ci)
            if ci > 0:
                norm2_flush()
                wo_proj(ci - 1)
            attention(ci)
        norm2_flush()
        wo_proj(N_CH - 1)


def _shard_inputs(x, Wq, bq, Wk, bk, Wv, bv, Wo, bo):
    mm_dt = _CACHE.get("mm_dt", "f16")
    ndt = np.float16 if mm_dt in ("f16in", "f16") else np.float32
    wdt = np.float16 if mm_dt == "f16" else np.float32
    x = np.asarray(x, np.float32)
    in_maps = []
    for core in range(N_CORES):
        b, g = divmod(core, 4)
        ds = slice(DG * g, DG * (g + 1))
        bqc = np.asarray(bq, np.float32)[ds].reshape(2, P).T
        bkc = np.asarray(bk, np.float32)[ds].reshape(2, P).T
        in_maps.append({
            "xT": np.ascontiguousarray(x[b].T).astype(ndt),
            "wq": np.ascontiguousarray(
                np.asarray(Wq, np.float32)[:, ds]).astype(ndt),
            "wk": np.ascontiguousarray(
                np.asarray(Wk, np.float32)[:, ds]).astype(ndt),
            "wv": np.ascontiguousarray(
                np.asarray(Wv, np.float32)[:, ds]).astype(ndt),
            "wo": np.ascontiguousarray(
                np.asarray(Wo, np.float32)[ds, :]).astype(wdt),
            "bias": np.ascontiguousarray(
                np.concatenate([bqc, bkc], axis=1)),
        })
    return in_maps


def kernel(x, Wq, bq, Wk, bk, Wv, bv, Wo, bo):
    mm_dt = _CACHE.get("mm_dt", "f16")
    _CACHE["mm_dt"] = mm_dt
    if "nc" not in _CACHE:
        _CACHE["nc"] = build_kernel(mm_dt)
    nc = _CACHE["nc"]
    in_maps = _shard_inputs(x, Wq, bq, Wk, bk, Wv, bv, Wo, bo)
    res = run_bass_kernel_spmd(
        nc, in_maps, core_ids=list(range(N_CORES)), trace=False)
    out = np.zeros((B, S, D), np.float32)
    for core in range(N_CORES):
        out[core // 4] += res.results[core]["o"]
    # exact bias folding: +bo, + bv @ Wo (constant row vector)
    out += (np.asarray(bo, np.float32)
            + np.asarray(bv, np.float32) @ np.asarray(Wo, np.float32))
    return out


# revision 9
# speedup vs baseline: 1.2795x; 1.0170x over previous
"""Causal multi-head attention on 8 trn2 NeuronCores.

Problem: B=2, S=2048, D=1024, H=16 heads, HD=64. fp32 in/out.

Sharding: 8 cores = 2 (batch) x 4 (head groups of 4 heads).
Each core computes, for its batch b and head group g:
  Q^T,K^T  [256, 2048]  (d on partitions, seq on free)  = W^T-slice x
  V        [2048, 256+ones]  (natural, with a ones column per head)
  per 512-wide q chunk, per head:  S^T[k,q] = K^T.T @ Q^T  (PE, contraction 64),
  P~ = exp(S^T/8) (ACT), causal via block skipping + one merged gpsimd
  affine_select per diagonal 2-block group, PV: ctx_raw^T[d,q] accumulated
  over k tiles with V_aug stationary (m=65; row 64 = softmax denominator).
  Normalize: DVE copies pv psum -> sbuf (releases the psum bank early),
  DVE reciprocal_approx_fast on the den row, gpsimd partition_broadcast,
  DVE mul -> ctx^T.  Then per chunk O_partial = ctx^T.T @ Wo_rows.
Host: sums the 4 head-group partials per batch and adds bo + bv @ Wo
(exact: the bv bias contributes the constant row vector bv @ Wo_g).

Emission order pipelines chunks: proj(ci) -> Wo(ci-1) -> attention(ci),
so the normalize chain of chunk ci-1 and Wo wait-time overlap the next
chunk's projections.  All input DMAs are single batched descriptors
(1 per weight tensor, 1 per x chunk); output stores are batched per
chunk and issued from the scalar-engine HWDGE queue so the sync queue
(input loads) is never blocked behind stores at the loop boundary.

Default mode "f16": ships x/Wq/Wk/Wv/Wo as fp16 and runs ALL matmuls in
native fp16 (1 cycle/row at any N, FWL fast weight loads) accumulating in
fp32 PSUM; p_t/qt/kt/vaug/ctxT/out in fp16 sbuf.  Normalization
internals (den, reciprocal, broadcast) in fp32.
"""

import math
import sys

if "/opt/trn_rl_repo" not in sys.path:
    sys.path.insert(0, "/opt/trn_rl_repo")

import numpy as np

import concourse.bacc as bacc
import concourse.bass as bass
import concourse.mybir as mybir
import concourse.tile as tile
from concourse.bass_utils import run_bass_kernel_spmd

B, S, D, H = 2, 2048, 1024, 16
HD = D // H  # 64
N_CORES = 8
HEADS_PER_CORE = H // 4  # 4
DG = HEADS_PER_CORE * HD  # 256 head dims per core
P = 128
CHUNK = 512  # q chunk width
N_KT = S // P  # 16 k tiles
N_CH = S // CHUNK  # 4 q chunks
F32 = mybir.dt.float32
F16 = mybir.dt.float16

_CACHE = {}


def _mm(dt_name):
    return {"f16": F16,
            "f32r": mybir.dt.float32r, "f16in": mybir.dt.float32r,
            "f32": mybir.dt.float32,
            "bf16": mybir.dt.bfloat16}[dt_name]


def _in_dt(dt_name):
    """dtype for the x / Wq / Wk / Wv inputs (DMA-traffic dominant)."""
    return F16 if dt_name in ("f16in", "f16") else _mm(dt_name)


def build_kernel(mm_dt="f16", unroll=1, ablate=()):
    """Build + compile the per-core SPMD program. unroll>1 wraps the body
    in a hardware loop (for pure device timing measurements)."""
    mdt = _mm(mm_dt)
    idt = _in_dt(mm_dt)
    odt = F16 if mm_dt == "f16" else F32

    nc = bacc.Bacc("TRN2", target_bir_lowering=False, debug=False)
    xT_d = nc.dram_tensor("xT", [D, S], idt, kind="ExternalInput")
    wq_d = nc.dram_tensor("wq", [D, DG], idt, kind="ExternalInput")
    wk_d = nc.dram_tensor("wk", [D, DG], idt, kind="ExternalInput")
    wv_d = nc.dram_tensor("wv", [D, DG], idt, kind="ExternalInput")
    wo_d = nc.dram_tensor("wo", [DG, D], mdt, kind="ExternalInput")
    # packed per-partition biases: col 0,1 = bq m-tiles, col 2,3 = bk m-tiles
    bias_d = nc.dram_tensor("bias", [P, 4], F32, kind="ExternalInput")
    o_d = nc.dram_tensor("o", [S, D], odt, kind="ExternalOutput")

    NDT = D // P  # 8 contraction tiles over D
    NMT = DG // P  # 2 m-tiles over the core's head dims (= head pairs)

    import contextlib
    with tile.TileContext(nc) as tc, contextlib.ExitStack() as stk:
        with tc.tile_pool(name="warm", bufs=1) as warm_p:
            # tiny pre-loop exp so the ACT table set is loaded once in the
            # preamble, not on every loop iteration
            wt = warm_p.tile([1, 8], F32, tag="warm", name="warm")
            nc.vector.memset(wt[:], 0.0)
            nc.scalar.activation(wt[:], wt[:],
                                 mybir.ActivationFunctionType.Exp)

        kb = _KernelBody(tc, nc, stk, mdt, idt, odt, xT_d, wq_d, wk_d,
                         wv_d, wo_d, bias_d, o_d, NDT, NMT, ablate)
        if unroll > 1:
            # software-pipelined loads: prologue loads before the loop;
            # each body reloads its inputs at the end (as soon as the last
            # reader of each tile retires) so transfers overlap the tail
            # and the next iteration starts computing immediately.
            kb.emit_loads()

            def body(_iv=None):
                kb.emit_compute()
                kb.emit_loads()

            with tc.For_i(0, unroll, 1):
                body()
        else:
            kb.emit_loads()
            kb.emit_compute()

    nc.compile()
    return nc


class _KernelBody:
    """Holds pools + persistent tiles; emits loads / compute separately so
    the loop can software-pipeline input loads against the compute tail."""

    def __init__(self, tc, nc, stk, mdt, idt, odt, xT_d, wq_d, wk_d, wv_d,
                 wo_d, bias_d, o_d, NDT, NMT, ablate=()):
        self.tc, self.nc = tc, nc
        self.mdt, self.idt, self.odt = mdt, idt, odt
        self.xT_d, self.wq_d, self.wk_d, self.wv_d = xT_d, wq_d, wk_d, wv_d
        self.wo_d, self.bias_d, self.o_d = wo_d, bias_d, o_d
        self.NDT, self.NMT = NDT, NMT
        self.ablate = ablate

        const = stk.enter_context(tc.tile_pool(name="const", bufs=1))
        sbuf = stk.enter_context(tc.tile_pool(name="sbuf", bufs=1))
        self.ptile_p = stk.enter_context(tc.tile_pool(name="ptile", bufs=8))
        self.craw_p = stk.enter_context(tc.tile_pool(name="craw", bufs=2))
        self.rec_p = stk.enter_context(tc.tile_pool(name="rec", bufs=2))
        self.recb_p = stk.enter_context(tc.tile_pool(name="recb", bufs=2))
        self.out_p = stk.enter_context(tc.tile_pool(name="outp", bufs=2))
        self.qkv_ps = stk.enter_context(
            tc.tile_pool(name="qkv_ps", bufs=2, space="PSUM"))
        self.stp_ps = stk.enter_context(
            tc.tile_pool(name="stp_ps", bufs=2, space="PSUM"))
        self.pv_ps = stk.enter_context(
            tc.tile_pool(name="pv_ps", bufs=2, space="PSUM"))

        self.xt = const.tile([P, NDT * S], idt, tag="xt", name="xt")
        self.ws = {name: const.tile([P, NDT * DG], idt, tag=name, name=name)
                   for name in ("wq", "wk", "wv")}
        self.wo = const.tile([P, NMT * D], mdt, tag="wo", name="wo")
        self.bias_t = const.tile([P, 4], F32, tag="bias", name="bias")
        self.biases = {("bq", 0): self.bias_t[:, 0:1],
                       ("bq", 1): self.bias_t[:, 1:2],
                       ("bk", 0): self.bias_t[:, 2:3],
                       ("bk", 1): self.bias_t[:, 3:4]}
        self.ones_r = const.tile([P, HEADS_PER_CORE], mdt, tag="ones_r",
                                 name="ones_r")
        nc.vector.memset(self.ones_r[:], 1.0)
        self.vaug = [sbuf.tile([P, HEADS_PER_CORE * (HD + 1)], mdt,
                               tag=f"vaug{j}", name=f"vaug{j}")
                     for j in range(N_KT)]
        self.qt = [sbuf.tile([P, S], mdt, tag=f"wqT{m}", name=f"wqT{m}")
                   for m in range(NMT)]
        self.kt = [sbuf.tile([P, S], mdt, tag=f"wkT{m}", name=f"wkT{m}")
                   for m in range(NMT)]
        self.ctxT = [sbuf.tile([P, S], mdt, tag=f"ctxT{m}", name=f"ctxT{m}")
                     for m in range(NMT)]

    # ---- input loads (batched descriptors, sync queue = loads only) -----
    def dma_w(self, name):
        d = {"wq": self.wq_d, "wk": self.wk_d, "wv": self.wv_d}[name]
        dst = self.ws[name][:].rearrange("p (k c) -> p k c", k=self.NDT)
        src = d.ap().rearrange("(k p) c -> p k c", k=self.NDT)
        self.nc.sync.dma_start(dst, src)

    def dma_xt(self, ci):
        csl = slice(CHUNK * ci, CHUNK * (ci + 1))
        dst = self.xt[:].rearrange("p (k s) -> p k s", k=self.NDT)[:, :, csl]
        src = self.xT_d.ap().rearrange("(k p) s -> p k s",
                                       k=self.NDT)[:, :, csl]
        self.nc.sync.dma_start(dst, src)

    def emit_loads(self):
        nc = self.nc
        self.dma_w("wv")
        self.dma_xt(0)
        self.dma_w("wq")
        self.dma_w("wk")
        nc.sync.dma_start(self.bias_t[:], self.bias_d.ap()[:, :])
        for ci in range(1, N_CH):
            self.dma_xt(ci)
        nc.sync.dma_start(
            self.wo[:].rearrange("p (m d) -> p m d", m=self.NMT),
            self.wo_d.ap().rearrange("(m p) d -> p m d", m=self.NMT))

    # ---- compute --------------------------------------------------------
    def xts(self, k, cols):
        return self.xt[:, S * k + cols.start:S * k + cols.stop]

    def wss(self, name, k, c0=0, c1=DG):
        return self.ws[name][:, DG * k + c0:DG * k + c1]

    def v_proj(self, j):
        nc = self.nc
        ps = self.qkv_ps.tile([P, CHUNK], F32, tag="proj", name="proj")
        for k in range(self.NDT):
            nc.tensor.matmul(
                ps[:, 0:DG],
                self.xts(k, slice(P * j, P * (j + 1))),
                self.wss("wv", k),
                start=(k == 0), stop=(k == self.NDT - 1))
        dst = self.vaug[j][:].rearrange("p (h x) -> p h x",
                                        h=HEADS_PER_CORE)
        srcp = ps[:, 0:DG].rearrange("p (h x) -> p h x", h=HEADS_PER_CORE)
        nc.vector.tensor_copy(dst[:, :, 0:HD], srcp[:, :, :])
        nc.vector.tensor_copy(
            dst[:, :, HD:HD + 1],
            self.ones_r[:].rearrange("p (h x) -> p h x", x=1))

    def qk_proj(self, ci):
        nc = self.nc
        for name, lst in (("wq", self.qt), ("wk", self.kt)):
            bname = "bq" if name == "wq" else "bk"
            for m in range(self.NMT):
                ps = self.qkv_ps.tile([P, CHUNK], F32, tag="proj",
                                      name="proj")
                for k in range(self.NDT):
                    nc.tensor.matmul(
                        ps[:],
                        self.wss(name, k, P * m, P * (m + 1)),
                        self.xts(k, slice(CHUNK * ci, CHUNK * (ci + 1))),
                        start=(k == 0), stop=(k == self.NDT - 1))
                nc.vector.tensor_scalar_add(
                    lst[m][:, CHUNK * ci:CHUNK * (ci + 1)], ps[:],
                    self.biases[(bname, m)])

    def wo_proj(self, ci):
        nc = self.nc
        ot = self.out_p.tile([P, 4 * D], self.odt, tag="ot", name="ot")
        for qi in range(4):
            i = 4 * ci + qi
            for e in range(2):
                ps = self.qkv_ps.tile([P, CHUNK], F32, tag="proj",
                                      name="proj")
                for m in range(self.NMT):
                    nc.tensor.matmul(
                        ps[:],
                        self.ctxT[m][:, P * i:P * (i + 1)],
                        self.wo[:, D * m + CHUNK * e:
                                D * m + CHUNK * (e + 1)],
                        start=(m == 0), stop=(m == self.NMT - 1))
                nc.any.tensor_copy(
                    ot[:, D * qi + CHUNK * e:D * qi + CHUNK * (e + 1)],
                    ps[:])
        # batched store for the whole chunk, issued from the scalar (ACT)
        # HWDGE queue: keeps the sync queue free for loads
        dst = self.o_d.ap()[CHUNK * ci:CHUNK * (ci + 1), :].rearrange(
            "(q p) d -> p q d", q=4)
        nc.scalar.dma_start(dst,
                            ot[:].rearrange("p (q d) -> p q d", q=4))

    def attention(self, ci):
        nc = self.nc
        ablate = self.ablate
        jmax = 4 * ci + 3  # last valid k tile for this chunk
        qsl = slice(CHUNK * ci, CHUNK * (ci + 1))
        for pair in range(self.NMT):
            pv = [self.pv_ps.tile([HD + 1, CHUNK], F32, tag="pv",
                                  name="pv") for _ in range(2)]
            for j0 in range(0, jmax + 1, 2):
                js = [j for j in (j0, j0 + 1) if j <= jmax]
                nj = len(js)
                pt = {}
                for hh in range(2):  # head within pair
                    psl = slice(64 * hh, 64 * (hh + 1))
                    st = self.stp_ps.tile([P, 2 * CHUNK], F32, tag="stp",
                                          name="stp")
                    for gi, j in enumerate(js):
                        nc.tensor.matmul(
                            st[:, CHUNK * gi:CHUNK * (gi + 1)],
                            self.kt[pair][psl, P * j:P * (j + 1)],
                            self.qt[pair][psl, qsl],
                            start=True, stop=True)
                    p_t = self.ptile_p.tile([P, 2 * CHUNK], self.mdt,
                                            tag="ptile", name="ptile")
                    if "exp" in ablate:
                        nc.vector.tensor_copy(
                            p_t[:, 0:CHUNK * nj], st[:, 0:CHUNK * nj])
                    else:
                        nc.scalar.activation(
                            p_t[:, 0:CHUNK * nj], st[:, 0:CHUNK * nj],
                            mybir.ActivationFunctionType.Exp,
                            scale=0.125)
                    if j0 >= 4 * ci and "mask" not in ablate:
                        # both js are diagonal blocks: one merged
                        # affine_select; keep where q >= k + 128*dd
                        dd0 = j0 - 4 * ci
                        sel = p_t[:, 0:CHUNK * nj].rearrange(
                            "p (b q) -> p b q", b=nj)
                        nc.gpsimd.affine_select(
                            out=sel,
                            in_=sel,
                            compare_op=mybir.AluOpType.is_ge,
                            fill=0.0, base=-P * dd0,
                            pattern=[[-P, nj], [1, CHUNK]],
                            channel_multiplier=-1)
                    pt[hh] = p_t
                for gi, j in enumerate(js):
                    for hh in range(2):
                        h = 2 * pair + hh
                        nc.tensor.matmul(
                            pv[hh][:],
                            self.vaug[j][:,
                                         (HD + 1) * h:(HD + 1) * (h + 1)],
                            pt[hh][:, CHUNK * gi:CHUNK * (gi + 1)],
                            start=(j == 0), stop=(j == jmax))
            # ---- normalize phase 1: copy psum out (releases the pv bank
            # early).  Phase 2 (recip/broadcast/mul) is deferred past the
            # next chunk's projections so it never blocks the qk
            # evacuations in DVE program order.
            if "div" in ablate:
                for hh in range(2):
                    nc.vector.tensor_copy(
                        self.ctxT[pair][64 * hh:64 * (hh + 1), qsl],
                        pv[hh][0:HD, :])
            else:
                for hh in range(2):
                    craw = self.craw_p.tile([HD, CHUNK], F32,
                                            tag=f"craw{hh}",
                                            name=f"craw{hh}")
                    nc.vector.tensor_copy(craw[:], pv[hh][0:HD, :])
                    den = self.rec_p.tile([1, CHUNK], F32, tag=f"den{hh}",
                                          name=f"den{hh}")
                    # partition-base-shifting copy (64 -> 0); custom DVE
                    # ops need partition-0-based operands
                    nc.vector.tensor_copy(den[0:1, :],
                                          pv[hh][HD:HD + 1, :])
                    self.norm2_work.append((pair, hh, qsl, craw, den))

    def norm2_flush(self):
        nc = self.nc
        while self.norm2_work:
            pair, hh, qsl, craw, den = self.norm2_work.pop(0)
            rec = self.rec_p.tile([1, CHUNK], F32, tag=f"rec{hh}",
                                  name=f"rec{hh}")
            nc.vector.reciprocal_approx_fast(rec[0:1, :], den[0:1, :])
            recb = self.recb_p.tile([HD, CHUNK], F32, tag=f"recb{hh}",
                                    name=f"recb{hh}")
            nc.gpsimd.partition_broadcast(recb[0:HD, :], rec[0:1, :])
            nc.vector.tensor_mul(
                self.ctxT[pair][64 * hh:64 * (hh + 1), qsl],
                craw[0:HD, :],
                recb[0:HD, :])

    def emit_compute(self):
        # pipelined emission: proj(ci) -> norm2+Wo(ci-1) -> attention(ci)
        self.norm2_work = []
        for ci in range(N_CH):
            for j in range(4 * ci, 4 * ci + 4):
                self.v_proj(j)
            self.qk_proj(ci)
            if ci > 0:
                self.norm2_flush()
                self.wo_proj(ci - 1)
            self.attention(ci)
        self.norm2_flush()
        self.wo_proj(N_CH - 1)


def _shard_inputs(x, Wq, bq, Wk, bk, Wv, bv, Wo, bo):
    mm_dt = _CACHE.get("mm_dt", "f16")
    ndt = np.float16 if mm_dt in ("f16in", "f16") else np.float32
    wdt = np.float16 if mm_dt == "f16" else np.float32
    x = np.asarray(x, np.float32)
    in_maps = []
    for core in range(N_CORES):
        b, g = divmod(core, 4)
        ds = slice(DG * g, DG * (g + 1))
        bqc = np.asarray(bq, np.float32)[ds].reshape(2, P).T
        bkc = np.asarray(bk, np.float32)[ds].reshape(2, P).T
        in_maps.append({
            "xT": np.ascontiguousarray(x[b].T).astype(ndt),
            "wq": np.ascontiguousarray(
                np.asarray(Wq, np.float32)[:, ds]).astype(ndt),
            "wk": np.ascontiguousarray(
                np.asarray(Wk, np.float32)[:, ds]).astype(ndt),
            "wv": np.ascontiguousarray(
                np.asarray(Wv, np.float32)[:, ds]).astype(ndt),
            "wo": np.ascontiguousarray(
                np.asarray(Wo, np.float32)[ds, :]).astype(wdt),
            "bias": np.ascontiguousarray(
                np.concatenate([bqc, bkc], axis=1)),
        })
    return in_maps


def kernel(x, Wq, bq, Wk, bk, Wv, bv, Wo, bo):
    mm_dt = _CACHE.get("mm_dt", "f16")
    _CACHE["mm_dt"] = mm_dt
    if "nc" not in _CACHE:
        _CACHE["nc"] = build_kernel(mm_dt)
    nc = _CACHE["nc"]
    in_maps = _shard_inputs(x, Wq, bq, Wk, bk, Wv, bv, Wo, bo)
    res = run_bass_kernel_spmd(
        nc, in_maps, core_ids=list(range(N_CORES)), trace=False)
    out = np.zeros((B, S, D), np.float32)
    for core in range(N_CORES):
        out[core // 4] += res.results[core]["o"]
    # exact bias folding: +bo, + bv @ Wo (constant row vector)
    out += (np.asarray(bo, np.float32)
            + np.asarray(bv, np.float32) @ np.asarray(Wo, np.float32))
    return out
